# revision 1
# baseline (speedup 1.0000x reference)
import sys

sys.path.insert(0, "/opt/trn_rl_repo")
import numpy as np
import ml_dtypes
import concourse.bass as bass
import concourse.mybir as mybir
import concourse.tile as tile
from concourse.bass_utils import run_bass_kernel_spmd

F32 = mybir.dt.float32
BF16 = mybir.dt.bfloat16
AF = mybir.ActivationFunctionType
ALU = mybir.AluOpType

C = 512
NH = 4          # heads per core (8 global, split in 2 groups of 4)
HD = 64
THETA = 10.0


import json as _json
import concourse.bass2jax as _b2j
import concourse.bass_utils as _bu

_ORIG_COMPILE = _bu.compile_bir_kernel


def _patched_compile_bir_kernel(bir_json, tmpdir, neff_name="file.neff"):
    """This walrus rejects instructions whose sync waits+updates exceed 2.
    Rewrite the BIR: move excess waits onto inserted same-engine Drains."""
    d = _json.loads(bir_json)
    for fn in d.get("functions", []):
        for b in fn.get("blocks", []):
            out = []
            for i in b.get("instructions", []):
                si = i.get("sync_info")
                if si:
                    ow = si.get("on_wait") or []
                    ou = si.get("on_update") or []
                    cap = 1 if i.get("opcode") == "Drain" else 2
                    budget = cap - len(ou)
                    if len(ow) > budget:
                        keep = ow[-budget:] if budget > 0 else []
                        extra = ow[:-budget] if budget > 0 else ow
                        for ci, w in enumerate(extra):
                            out.append({
                                "debug": i.get("debug", 0),
                                "engine": i["engine"],
                                "ins": [], "outs": [],
                                "name": f"{i['name']}sw{ci}",
                                "opcode": "Drain",
                                "sync_info": {"on_update": [],
                                              "on_wait": [w]},
                            })
                        si["on_wait"] = keep
                out.append(i)
            b["instructions"] = out
    return _ORIG_COMPILE(_json.dumps(d).encode(), tmpdir, neff_name=neff_name)


_bu.compile_bir_kernel = _patched_compile_bir_kernel
_b2j.compile_bir_kernel = _patched_compile_bir_kernel


def _build_nc(n_tok):
    nspan = n_tok // 512
    nc = bass.Bass()
    xT = nc.declare_dram_parameter("xT", [C, n_tok], BF16, isOutput=False)
    w_qk = nc.declare_dram_parameter("w_qk", [C, 512], BF16, isOutput=False)
    w_v = nc.declare_dram_parameter("w_v", [C, 256], BF16, isOutput=False)
    brow = nc.declare_dram_parameter("brow", [1, 512], BF16, isOutput=False)
    wp = nc.declare_dram_parameter("wp", [2, 128, 512], BF16, isOutput=False)
    cosD = nc.declare_dram_parameter("cosD", [n_tok, 256], BF16, isOutput=False)
    sinD = nc.declare_dram_parameter("sinD", [n_tok, 256], BF16, isOutput=False)
    ident = nc.declare_dram_parameter("ident", [128, 128], BF16, isOutput=False)
    exp2 = nc.declare_dram_parameter("exp2", [2, 128], BF16, isOutput=False)
    y = nc.declare_dram_parameter("y", [n_tok, 512], F32, isOutput=True)

    with nc.allow_low_precision(reason="bf16 pipeline by design"), tile.TileContext(nc) as tc:
        with tc.tile_pool(name="wpool", bufs=1) as wpool, \
             tc.tile_pool(name="store", bufs=1) as store:
            wqk_t = [wpool.tile([128, 512], BF16, name=f"wqk{c}") for c in range(4)]
            wv_t = [wpool.tile([128, 256], BF16, name=f"wv{c}") for c in range(4)]
            br_t = wpool.tile([1, 512], BF16, name="br")
            ones_t = wpool.tile([1, 128], BF16, name="ones")
            id_t = wpool.tile([128, 128], BF16, name="id")
            e2_t = wpool.tile([34, 128], BF16, name="e2")
            wp_t = [wpool.tile([128, 512], BF16, name=f"wp{i}") for i in range(2)]
            for c in range(4):
                nc.sync.dma_start(wqk_t[c][:], w_qk[c * 128:(c + 1) * 128, :])
                nc.sync.dma_start(wv_t[c][:], w_v[c * 128:(c + 1) * 128, :])
            nc.sync.dma_start(br_t[:], brow[:])
            nc.sync.dma_start(id_t[:], ident[:])
            nc.sync.dma_start(e2_t[0:2, :], exp2[:])
            nc.sync.dma_start(e2_t[32:34, :], exp2[:])
            for i in range(2):
                nc.sync.dma_start(wp_t[i][:], wp[i])
            nc.vector.memset(ones_t[:], 1.0)

            q_store = store.tile([128, (n_tok // 128) * 256], BF16, name="qs")
            lR = [wpool.tile([128, 128], BF16, name=f"lR{i}") for i in range(2)]
            lI = [wpool.tile([128, 128], BF16, name=f"lI{i}") for i in range(2)]
            denR = wpool.tile([128, 34], BF16, name="denR")
            denI = wpool.tile([128, 34], BF16, name="denI")

            # ================ pass 1 ================
            with tc.tile_pool(name="kvps", bufs=1, space="PSUM") as kvps:
                kvR = kvps.tile([128, 129], F32, name="kvR")
                kvI = kvps.tile([128, 129], F32, name="kvI")
                with tc.tile_pool(name="p1", bufs=2) as p1, \
                     tc.tile_pool(name="xp", bufs=8) as xp, \
                     tc.tile_pool(name="ps1", bufs=1, space="PSUM") as ps1:
                    co = cosD.rearrange("(s t p) c -> s p t c", p=128, t=4)
                    si = sinD.rearrange("(s t p) c -> s p t c", p=128, t=4)
                    for s in range(nspan):
                        xt = [xp.tile([128, 512], BF16, name="xt") for _ in range(4)]
                        for c in range(4):
                            nc.sync.dma_start(
                                xt[c][:], xT[c * 128:(c + 1) * 128, s * 512:(s + 1) * 512])
                        cos_t = p1.tile([128, 4, 256], BF16, name="cos")
                        sin_t = p1.tile([128, 4, 256], BF16, name="sin")
                        nc.sync.dma_start(cos_t[:], co[s])
                        nc.sync.dma_start(sin_t[:], si[s])

                        qk_ps = ps1.tile([128, 4, 512], F32, name="qk")
                        v_ps = ps1.tile([128, 4, 256], F32, name="v")
                        for t in range(4):
                            for c in range(4):
                                nc.tensor.matmul(
                                    qk_ps[:, t, :], xt[c][:, t * 128:(t + 1) * 128],
                                    wqk_t[c][:], start=(c == 0), stop=False)
                            nc.tensor.matmul(qk_ps[:, t, :], ones_t[:], br_t[:],
                                             start=False, stop=True)
                            for c in range(4):
                                nc.tensor.matmul(
                                    v_ps[:, t, :], xt[c][:, t * 128:(t + 1) * 128],
                                    wv_t[c][:], start=(c == 0), stop=(c == 3))

                        qk_sb = p1.tile([128, 4, 2, 2, 128], BF16, name="qksb")
                        nc.scalar.copy(
                            qk_sb[:],
                            qk_ps[:].rearrange("p t (g a c) -> p t g a c", g=2, a=2))
                        cg = cos_t[:].rearrange("p t (g c) -> p t g c", g=2)
                        sg = sin_t[:].rearrange("p t (g c) -> p t g c", g=2)
                        RR = qk_sb[:, :, :, 0, :]
                        II = qk_sb[:, :, :, 1, :]
                        t1 = p1.tile([128, 4, 2, 128], BF16, name="t1")
                        t2 = p1.tile([128, 4, 2, 128], BF16, name="t2")
                        t3 = p1.tile([128, 4, 2, 128], BF16, name="t3")
                        t4 = p1.tile([128, 4, 2, 128], BF16, name="t4")
                        nc.vector.tensor_tensor(t1[:], RR, cg, op=ALU.mult)
                        nc.vector.tensor_tensor(t2[:], II, cg, op=ALU.mult)
                        nc.vector.tensor_tensor(t3[:], RR, sg, op=ALU.mult)
                        nc.vector.tensor_tensor(t4[:], II, sg, op=ALU.mult)

                        qsv = q_store[:, s * 1024:(s + 1) * 1024].rearrange(
                            "p (t a c) -> p t a c", t=4, a=2)
                        kf = p1.tile([128, 4, 2, 128], BF16, name="kf")
                        nc.vector.tensor_tensor(qsv[:, :, 0, :], t1[:, :, 0, :],
                                                t4[:, :, 0, :], op=ALU.subtract)
                        nc.vector.tensor_tensor(kf[:, :, 0, :], t1[:, :, 1, :],
                                                t4[:, :, 1, :], op=ALU.subtract)
                        nc.vector.tensor_tensor(qsv[:, :, 1, :], t3[:, :, 0, :],
                                                t2[:, :, 0, :], op=ALU.add)
                        nc.vector.tensor_tensor(kf[:, :, 1, :], t3[:, :, 1, :],
                                                t2[:, :, 1, :], op=ALU.add)

                        # elu(x)+1 = relu(x) + exp(min(x,0))
                        qs2 = q_store[:, s * 1024:(s + 1) * 1024].rearrange(
                            "p (t c) -> p t c", t=4)
                        kf2 = kf[:].rearrange("p t a c -> p t (a c)")
                        for src in (qs2, kf2):
                            m = p1.tile([128, 4, 256], BF16, name="elm")
                            e = p1.tile([128, 4, 256], BF16, name="ele")
                            r = p1.tile([128, 4, 256], BF16, name="elr")
                            nc.vector.tensor_scalar_min(m[:], src, 0.0)
                            nc.scalar.activation(e[:], m[:], AF.Exp)
                            nc.scalar.activation(r[:], src, AF.Relu)
                            nc.vector.tensor_tensor(src, e[:], r[:], op=ALU.add)

                        v_sb = p1.tile([128, 4, 258], BF16, name="vsb")
                        nc.vector.memset(v_sb[:], 1.0)
                        nc.scalar.copy(v_sb[:, :, 0:128], v_ps[:, :, 0:128])
                        nc.scalar.copy(v_sb[:, :, 129:257], v_ps[:, :, 128:256])

                        first, last = (s == 0), (s == nspan - 1)
                        for t in range(4):
                            st, sp = (first and t == 0), (last and t == 3)
                            nc.tensor.matmul(kvR[0:64, :], kf2[:, t, 0:64],
                                             v_sb[:, t, 0:129], start=st, stop=sp)
                            nc.tensor.matmul(kvI[0:64, :], kf2[:, t, 128:192],
                                             v_sb[:, t, 0:129], start=st, stop=sp)
                            nc.tensor.matmul(kvR[64:128, :], kf2[:, t, 64:128],
                                             v_sb[:, t, 129:258], start=st, stop=sp)
                            nc.tensor.matmul(kvI[64:128, :], kf2[:, t, 192:256],
                                             v_sb[:, t, 129:258], start=st, stop=sp)

                # kv psum -> block-diag lhsT tiles + denom columns
                for tl in lR + lI + [denR, denI]:
                    nc.vector.memset(tl[:], 0.0)
                for i, lo in enumerate((0, 64)):
                    nc.scalar.copy(lR[i][lo:lo + 32, 0:64], kvR[lo:lo + 32, 0:64])
                    nc.scalar.copy(lR[i][lo + 32:lo + 64, 64:128], kvR[lo + 32:lo + 64, 64:128])
                    nc.scalar.copy(lI[i][lo:lo + 32, 0:64], kvI[lo:lo + 32, 0:64])
                    nc.scalar.copy(lI[i][lo + 32:lo + 64, 64:128], kvI[lo + 32:lo + 64, 64:128])
                for j in range(4):
                    col = j if j < 2 else 32 + (j - 2)
                    nc.scalar.copy(denR[j * 32:(j + 1) * 32, col:col + 1],
                                   kvR[j * 32:(j + 1) * 32, 128:129])
                    nc.scalar.copy(denI[j * 32:(j + 1) * 32, col:col + 1],
                                   kvI[j * 32:(j + 1) * 32, 128:129])

            # ================ pass 2 ================
            with tc.tile_pool(name="p2", bufs=2) as p2, \
                 tc.tile_pool(name="ps2", bufs=1, space="PSUM") as ps2, \
                 tc.tile_pool(name="psy", bufs=1, space="PSUM") as psy:
                for s in range(nspan):
                    qTa = ps2.tile([128, 512], BF16, name="qTa")
                    qTb = ps2.tile([128, 512], BF16, name="qTb")
                    for t in range(4):
                        base = (4 * s + t) * 256
                        nc.tensor.transpose(qTa[:, t * 128:(t + 1) * 128],
                                            q_store[:, base:base + 128], id_t[:])
                        nc.tensor.transpose(qTb[:, t * 128:(t + 1) * 128],
                                            q_store[:, base + 128:base + 256], id_t[:])
                    qa_sb = p2.tile([128, 512], BF16, name="qa")
                    qb_sb = p2.tile([128, 512], BF16, name="qb")
                    nc.scalar.copy(qa_sb[:], qTa[:])
                    nc.scalar.copy(qb_sb[:], qTb[:])

                    out_ps = [ps2.tile([128, 512], F32, name=f"o{i}") for i in range(2)]
                    den_ps = ps2.tile([64, 512], F32, name="den")
                    for i in range(2):
                        nc.tensor.matmul(out_ps[i][:], lR[i][:], qa_sb[:], start=True, stop=False)
                        nc.tensor.matmul(out_ps[i][:], lI[i][:], qb_sb[:], start=False, stop=True)
                    nc.tensor.matmul(den_ps[0:34, :], denR[:], qa_sb[:], start=True, stop=False)
                    nc.tensor.matmul(den_ps[0:34, :], denI[:], qb_sb[:], start=False, stop=True)

                    zr = p2.tile([64, 512], F32, name="zr")
                    zb = p2.tile([64, 512], BF16, name="zb")
                    nc.vector.tensor_scalar_add(zr[0:34, :], den_ps[0:34, :], 1e-6)
                    nc.vector.reciprocal(zb[0:34, :], zr[0:34, :])
                    zb_ps = [ps2.tile([128, 512], F32, name=f"zp{i}") for i in range(2)]
                    zb_sb = [p2.tile([128, 512], BF16, name=f"zs{i}") for i in range(2)]
                    outT = [p2.tile([128, 512], BF16, name=f"oT{i}") for i in range(2)]
                    for i in range(2):
                        nc.tensor.matmul(zb_ps[i][:], e2_t[32 * i:32 * i + 2, :], zb[32 * i:32 * i + 2, :],
                                         start=True, stop=True)
                        nc.scalar.copy(zb_sb[i][:], zb_ps[i][:])
                        nc.vector.tensor_tensor(outT[i][:], out_ps[i][:], zb_sb[i][:],
                                                op=ALU.mult)

                    for t in range(4):
                        y_ps = psy.tile([128, 512], F32, name="y")
                        nc.tensor.matmul(y_ps[:], outT[0][:, t * 128:(t + 1) * 128],
                                         wp_t[0][:], start=True, stop=False)
                        nc.tensor.matmul(y_ps[:], outT[1][:, t * 128:(t + 1) * 128],
                                         wp_t[1][:], start=False, stop=True)
                        y_sb = p2.tile([128, 512], F32, name="ysb")
                        nc.scalar.copy(y_sb[:], y_ps[:])
                        nc.sync.dma_start(
                            y[s * 512 + t * 128: s * 512 + (t + 1) * 128, :], y_sb[:])

    return nc


_NC_CACHE = {}


def _get_nc(n_tok):
    if n_tok not in _NC_CACHE:
        _NC_CACHE[n_tok] = _build_nc(n_tok)
    return _NC_CACHE[n_tok]


def _rope_tables(n, height, width):
    hd4 = HD // 4
    freqs = 1.0 / (THETA ** (np.arange(0, HD, 4)[:hd4].astype(np.float32) / HD))
    t = np.arange(n)
    t_x = (t % width).astype(np.float32)
    t_y = (t // width).astype(np.float32)
    ang_x = np.outer(t_x, freqs)
    ang_y = np.outer(t_y, freqs)
    base_c = np.concatenate([np.cos(ang_x), np.cos(ang_y)], axis=1)
    base_s = np.concatenate([np.sin(ang_x), np.sin(ang_y)], axis=1)
    return np.tile(base_c, (1, 8)), np.tile(base_s, (1, 8))


def _bf(a):
    return np.ascontiguousarray(np.asarray(a, dtype=np.float32)).astype(ml_dtypes.bfloat16)


def kernel(x, w_qkv, b_qkv, w_proj, b_proj, height, width):
    x = np.asarray(x); w_qkv = np.asarray(w_qkv); b_qkv = np.asarray(b_qkv)
    w_proj = np.asarray(w_proj); b_proj = np.asarray(b_proj)
    height = int(height); width = int(width)
    b, n, c = x.shape
    nc = _get_nc(n)
    cosD, sinD = _rope_tables(n, height, width)
    e2 = np.zeros((2, 128), np.float32)
    e2[0, 0:64] = 1.0
    e2[1, 64:128] = 1.0

    in_maps = []
    for core in range(8):
        bi, hg = core // 2, core % 2
        heads = [hg * NH + j for j in range(NH)]
        qR = [h * HD + 2 * s for h in heads for s in range(32)]
        qI = [h * HD + 2 * s + 1 for h in heads for s in range(32)]
        kR = [512 + h * HD + 2 * s for h in heads for s in range(32)]
        kI = [512 + h * HD + 2 * s + 1 for h in heads for s in range(32)]
        vc = [1024 + h * HD + e for h in heads for e in range(HD)]
        in_maps.append({
            "xT": _bf(x[bi].T),
            "w_qk": _bf(w_qkv[:, qR + qI + kR + kI]),
            "w_v": _bf(w_qkv[:, vc]),
            "brow": _bf(b_qkv[qR + qI + kR + kI][None, :]),
            "wp": _bf(np.stack([w_proj[hg * 256:hg * 256 + 128, :],
                                w_proj[hg * 256 + 128:hg * 256 + 256, :]])),
            "cosD": _bf(cosD), "sinD": _bf(sinD),
            "ident": _bf(np.eye(128, dtype=np.float32)), "exp2": _bf(e2),
        })
    res = run_bass_kernel_spmd(nc, in_maps, list(range(8)), trace=False)
    bias_eff = (b_proj.astype(np.float64)
                + b_qkv[1024:].astype(np.float64) @ w_proj.astype(np.float64))
    out = np.empty((b, n, c), np.float32)
    for bi in range(b):
        out[bi] = (res.results[2 * bi]["y"].astype(np.float64)
                   + res.results[2 * bi + 1]["y"].astype(np.float64)
                   + bias_eff[None, :]).astype(np.float32)
    return out

